# revision 24
# baseline (speedup 1.0000x reference)
"""Multi-head self-attention kernel for 8 Trainium2 NeuronCores.

Sharding: core c = (b, g) with b = batch index (4), g = head-group (2).
Each core computes attention for one batch element and 8 of the 16 heads,
including its slice of the QKV projections and a partial out-projection
(Y_partial = O_heads @ Wo[rows of its heads]).  The host sums the two
head-group partials per batch and transposes (the device produces Y^T).

On-device layout is fully "transposed": x^T [D, S] in, Q^T/K^T [dk, S],
scores S^T = K_h Q_h^T [k, q] (softmax along partitions via a ones-column
appended to V: the PV matmul O^T_aug = [V|1]^T P^T yields the softmax
denominator in its last row), output Y^T [D, S].

v2 design notes (the ACT exp stream and the PE matmul stream are nearly
balanced at ~266us / ~272us per core, so the schedule aims to keep both
saturated):
- all matmul operands in bf16; PSUM accumulation fp32; biases in fp32.
- the two per-pair score matmuls land in adjacent PSUM banks of one
  [P, 4, QB] tile (kc-alternating slot pairs) so a single Exp activation
  covers [P, 2*QB], halving per-instruction ACT overhead.
- the kc sweep is software-pipelined: the PE stream orders the next
  iteration's score matmuls BEFORE the current PV matmuls, so the PE
  never queues behind the exp dependency.
- Q-projection and out-projection matmuls are spread as "filler" into
  sweep iterations (2 per kc step) instead of blocking at boundaries,
  which would starve the exp stream.
- phase 1 (K^T/V projection) is merged with the (qb=0, pair=0) attention
  sweep so the ACT engine starts exp work as soon as window 0 exists.
- oa/ob PSUM accumulators are evacuated to SBUF by DVE immediately after
  the sweep so their banks free for the next pair; softmax normalization
  runs from the SBUF copies (reciprocal + gpsimd partition_broadcast).
"""

import sys

sys.path.insert(0, "/opt/trn_rl_repo")

from collections import deque
from contextlib import ExitStack

import numpy as np

import concourse.bass as bass
import concourse.tile as tile
from concourse import bacc, mybir
from concourse.bass_utils import run_bass_kernel_spmd

F32 = mybir.dt.float32
BF16 = mybir.dt.bfloat16
P = 128  # SBUF partitions

D_MODEL = 1024
NHEAD = 16
DK = D_MODEL // NHEAD  # 64
BATCH = 4
SEQ = 2048
N_CORES = 8
HL = NHEAD // 2  # heads per core (head-group of 8)


def build_bass(D=D_MODEL, S=SEQ, HLOC=HL, QB=512, repeat=1,
               qtpb=3, pexpb=3, fcap=2):
    """Build the per-core Bass program (same program on all 8 cores)."""
    DC = D // P           # d_model chunks (contraction for projections)
    KC = S // P           # key chunks
    NQB = S // QB         # q blocks
    NPAIR = HLOC // 2     # head pairs
    HD = HLOC * DK        # local head dim total (512)
    VW = DK + 1           # V columns per head incl. ones column
    NOC = D // P          # out-dim chunks
    EXP_SCALE = 1.0 / np.sqrt(DK)
    cfg = dict(D=D, S=S, HLOC=HLOC, QB=QB, DC=DC, KC=KC, NQB=NQB,
               NPAIR=NPAIR, HD=HD, VW=VW, NOC=NOC, EXP_SCALE=EXP_SCALE,
               fcap=fcap)

    nc = bacc.Bacc("TRN2", target_bir_lowering=False, debug=False,
                   num_devices=N_CORES)

    xT = nc.dram_tensor("xT", [D, S], BF16, kind="ExternalInput")
    Wq = nc.dram_tensor("Wq", [D, HD], BF16, kind="ExternalInput")
    Wk = nc.dram_tensor("Wk", [D, HD], BF16, kind="ExternalInput")
    Wv = nc.dram_tensor("Wv", [D, HD], BF16, kind="ExternalInput")
    Wo = nc.dram_tensor("Wo", [HD, D], BF16, kind="ExternalInput")
    bq_t = nc.dram_tensor("bq_t", [P, NPAIR], F32, kind="ExternalInput")
    bk_t = nc.dram_tensor("bk_t", [P, NPAIR], F32, kind="ExternalInput")
    bv_bc = nc.dram_tensor("bv_bc", [P, HD], F32, kind="ExternalInput")
    bo_t = nc.dram_tensor("bo_t", [P, NOC], F32, kind="ExternalInput")
    YT = nc.dram_tensor("YT", [D, S], F32, kind="ExternalOutput")
    dram = dict(xT=xT, Wq=Wq, Wk=Wk, Wv=Wv, Wo=Wo, bq_t=bq_t, bk_t=bk_t,
                bv_bc=bv_bc, bo_t=bo_t, YT=YT)

    with tile.TileContext(nc) as tc, ExitStack() as ctx:
        consts = ctx.enter_context(tc.tile_pool(name="consts", bufs=1))
        ktv = ctx.enter_context(tc.tile_pool(name="ktv", bufs=1))
        wper = ctx.enter_context(tc.tile_pool(name="wper", bufs=1))
        xres = ctx.enter_context(tc.tile_pool(name="xres", bufs=1))
        ps_s = ctx.enter_context(tc.tile_pool(name="ps_s", bufs=1,
                                              space="PSUM"))
        ps_b = ctx.enter_context(tc.tile_pool(name="ps_b", bufs=2, space="PSUM"))
        ps_acc = ctx.enter_context(tc.tile_pool(name="ps_acc", bufs=2,
                                                space="PSUM"))

        # ---- constants ----
        bq_sb = consts.tile([P, NPAIR], F32, tag="bq")
        bk_sb = consts.tile([P, NPAIR], F32, tag="bk")
        bv_sb = consts.tile([P, HD], F32, tag="bv")
        bo_sb = consts.tile([P, NOC], F32, tag="bo")
        nc.gpsimd.dma_start(bq_sb[:], bq_t.ap())
        nc.gpsimd.dma_start(bk_sb[:], bk_t.ap())
        nc.gpsimd.dma_start(bv_sb[:], bv_bc.ap())
        nc.gpsimd.dma_start(bo_sb[:], bo_t.ap())

        # warm the ACT exp table early
        warm = consts.tile([1, 2], F32, tag="warm")
        nc.gpsimd.memset(warm[0:1, 0:1], 0.0)
        nc.scalar.activation(warm[0:1, 1:2], warm[0:1, 0:1],
                             mybir.ActivationFunctionType.Exp)

        ones_sb = consts.tile([P, HLOC], BF16, tag="ones")
        nc.vector.memset(ones_sb[:], 1.0)

        sbs = dict(bq=bq_sb, bk=bk_sb, bv=bv_sb, bo=bo_sb, ones=ones_sb)
        pools = dict(consts=consts, ktv=ktv, wper=wper, xres=xres,
                     ps_s=ps_s, ps_b=ps_b, ps_acc=ps_acc)

        for _rep in range(repeat):
            emit_body(nc, tc, cfg, dram, sbs, pools, qtpb=qtpb, pexpb=pexpb)

    nc.compile()
    return nc


def emit_body(nc, tc, cfg, dram, sbs, pools, qtpb=3, pexpb=3):
    D, S, HLOC, QB = cfg["D"], cfg["S"], cfg["HLOC"], cfg["QB"]
    DC, KC, NQB, NPAIR = cfg["DC"], cfg["KC"], cfg["NQB"], cfg["NPAIR"]
    HD, VW, NOC, EXP_SCALE = cfg["HD"], cfg["VW"], cfg["NOC"], cfg["EXP_SCALE"]
    FCAP = cfg["fcap"]
    ktv, wper, xres = pools["ktv"], pools["wper"], pools["xres"]
    ps_s, ps_b, ps_acc = pools["ps_s"], pools["ps_b"], pools["ps_acc"]
    bq_sb, bk_sb, bv_sb = sbs["bq"], sbs["bk"], sbs["bv"]
    bo_sb, ones_sb = sbs["bo"], sbs["ones"]

    xt_dram3 = dram["xT"].ap().rearrange("(c p) s -> p c s", p=P)
    yt_dram3 = dram["YT"].ap().rearrange("(n p) s -> p n s", p=P)

    # resident tensors.  NOTE: the Tile framework tracks dependencies at
    # TILE granularity, so anything that must pipeline independently gets
    # its own tile (per-window x, per-parity score PSUM, per-chunk V).
    kt_tiles = [ktv.tile([P, S], BF16, tag=f"kt{p_}", name=f"kt{p_}")
                for p_ in range(NPAIR)]
    v_tiles = [ktv.tile([P, HLOC * VW], BF16, tag=f"v{k}", name=f"v{k}")
               for k in range(KC)]
    xt_w = [xres.tile([P, DC * QB], BF16, tag=f"xt{w}", name=f"xt{w}")
            for w in range(NQB)]
    xt3_w = [t[:].rearrange("p (c s) -> p c s", c=DC) for t in xt_w]

    # score/exp PSUM: two [P, 2, QB] tiles alternating by kc parity; one
    # Exp reads a full tile [P, 2*QB] while the next iteration's score
    # matmuls fill the other tile.  Separate tiles keep the WAR chain
    # exact: exp(kc) only waits on ITS scores, not the 2-ahead pair.
    s_par = [ps_s.tile([P, 2, QB], F32, tag=f"s{i}", name=f"s{i}")
             for i in range(2)]

    def wslice(wt, c, lo, hi):
        return wt[:, c * HD + lo: c * HD + hi]

    def load_w(pool, name, d, cols, eng=None):
        t = pool.tile([P, DC * cols], BF16, tag=name, name=name)
        (eng or nc.sync).dma_start(
            t[:].rearrange("p (c n) -> p c n", c=DC),
            d.ap().rearrange("(c p) n -> p c n", p=P))
        return t

    def load_w_pair(pool, name, d, pr):
        """One head-pair's 128 columns of a [D, HD] weight — a separate
        tile per pair so early consumers don't wait the whole matrix."""
        t = pool.tile([P, DC * P], BF16, tag=f"{name}{pr}",
                      name=f"{name}{pr}")
        nc.sync.dma_start(
            t[:].rearrange("p (c n) -> p c n", c=DC),
            d.ap().rearrange("(c p) n -> p c n", p=P)[:, :,
                                                      pr * P:(pr + 1) * P])
        return t

    # PE filler machinery: a queue of item-deques; each item is a closure
    # emitting ONE PE matmul (plus any trailing cheap non-PE ops).  Sweeps
    # pop up to FCAP items per kc step so projection work rides in the
    # exp-wait slack instead of blocking at pair/qb boundaries.  An item
    # deque can also be force-drained (e.g. a pair's Q^T projection must
    # be fully emitted before that pair's sweep reads the tile).
    filler_q = deque()

    def pop_filler(k=None):
        n = FCAP if k is None else k
        for _ in range(n):
            while filler_q and not filler_q[0]:
                filler_q.popleft()
            if not filler_q:
                return
            filler_q[0].popleft()()

    def queue_items(items, front=False):
        if front:
            filler_q.appendleft(items)
        else:
            filler_q.append(items)

    def force(items):
        while items:
            items.popleft()()

    def drain_filler():
        while filler_q:
            force(filler_q.popleft())

    def queue_qt_proj(qb, pr, front=False):
        """Queue the 8 projection matmuls for Q^T of (qb, pr); returns the
        destination tile and its item deque (force before first read).
        A q-block coincides with an x window (QB == window size)."""
        qps = ps_b.tile([P, QB], F32, tag="sp", name="qps")
        qt = qtp.tile([P, QB], BF16, tag="qt", name="qt")
        items = deque()
        for c in range(DC):
            def mm(c=c):
                nc.tensor.matmul(qps[:],
                                 wq_p[pr][:, c * P:(c + 1) * P],
                                 xt3_w[qb][:, c, :],
                                 start=(c == 0), stop=(c == DC - 1))
                if c == DC - 1:
                    nc.vector.tensor_scalar_add(qt[:], qps[:],
                                                bq_sb[:, pr:pr + 1])
            items.append(mm)
        if front:
            filler_q.appendleft(items)
        else:
            filler_q.append(items)
        return qt, items

    def queue_out_proj(qb, ot_tiles):
        """Queue the 32 out-projection matmuls + bias/DMA for q-block qb."""
        qsl = bass.ts(qb, QB)
        items = deque()
        for n in range(NOC):
            yps = ps_b.tile([P, QB], F32, tag="sp", name="yps")
            for pr in range(NPAIR):
                def mm(n=n, pr=pr, yps=yps):
                    nc.tensor.matmul(
                        yps[:],
                        wo_sb[:, pr * D + n * P: pr * D + (n + 1) * P],
                        ot_tiles[pr][:],
                        start=(pr == 0), stop=(pr == NPAIR - 1))
                    if pr == NPAIR - 1:
                        ysb = misc.tile([P, QB], F32, tag="ysb", name="ysb")
                        nc.vector.tensor_scalar_add(ysb[:], yps[:],
                                                    bo_sb[:, n:n + 1])
                        nc.sync.dma_start(yt_dram3[:, n, qsl], ysb[:])
                items.append(mm)
        filler_q.append(items)

    def scores(kt, qt, kc):
        ksl = bass.ts(kc, P)
        st = s_par[kc % 2]
        nc.tensor.matmul(st[:, 0, :], kt[0:DK, ksl],
                         qt[0:DK, :], start=True, stop=True)
        nc.tensor.matmul(st[:, 1, :], kt[DK:P, ksl],
                         qt[DK:P, :], start=True, stop=True)

    def exp_act(kc):
        """The Exp activation for chunk kc.  MUST be emitted before the
        same-parity scores(kc+2) so the write-after-read on the score tile
        is tracked in the correct direction."""
        st = s_par[kc % 2]
        e = pexp.tile([P, 2, QB], BF16, tag="e", name="e")
        nc.scalar.activation(e[:].rearrange("p a b -> p (a b)"),
                             st[:].rearrange("p a b -> p (a b)"),
                             mybir.ActivationFunctionType.Exp,
                             scale=float(EXP_SCALE))
        return e

    def pv(pr, kc, e, oa, ob):
        vt = v_tiles[kc]
        ha, hb = 2 * pr, 2 * pr + 1
        nc.tensor.matmul(oa[:], vt[:, ha * VW:(ha + 1) * VW],
                         e[:, 0, :], start=(kc == 0), stop=(kc == KC - 1))
        nc.tensor.matmul(ob[:], vt[:, hb * VW:(hb + 1) * VW],
                         e[:, 1, :], start=(kc == 0), stop=(kc == KC - 1))

    def sweep(kt, qt, pr, oa, ob, kc_lo, kc_hi, lookahead=(),
              prologue=True, cap=None, filler_first=False):
        """Software-pipelined attention sweep over key chunks [kc_lo, kc_hi).

        Scores run TWO steps ahead of the PV consumption: the critical
        timing cycle is exp(i) ack -> PV(i) -> scores(next) -> exp(next),
        so scores(i+1) must already sit in the PE FIFO before PV(i-1).
        `lookahead` supplies up to two score-emitting closures for the
        steps beyond kc_hi (the next pair's first chunks); `prologue`
        emits this sweep's first two score pairs (skip when the previous
        sweep's lookahead already emitted them).  `filler_first` emits the
        filler BEFORE the (SEQ-stalling) score matmuls — right for the
        PE-bound phase 1 where exp cadence doesn't bind; in the ACT-bound
        phase 2 filler goes after scores so they complete earlier."""
        if prologue:
            scores(kt, qt, kc_lo)
            if kc_lo + 1 < kc_hi:
                scores(kt, qt, kc_lo + 1)
        for kc in range(kc_lo, kc_hi):
            e = exp_act(kc)  # emit FIRST: read-before-overwrite on s_par
            if filler_first:
                pop_filler(cap)
            tgt = kc + 2
            if tgt < kc_hi:
                scores(kt, qt, tgt)
            elif tgt - kc_hi < len(lookahead):
                lookahead[tgt - kc_hi]()
            if not filler_first:
                pop_filler(cap)
            pv(pr, kc, e, oa, ob)

    def normalize(oa, ob):
        """Evacuate oa/ob to SBUF (frees their PSUM banks early), then
        normalize rows 0:DK by row DK (the ones-column sums)."""
        oc = misc.tile([VW, QB], F32, tag="oca", name="oc")
        od = misc.tile([VW, QB], F32, tag="ocb", name="od")
        nc.vector.tensor_copy(oc[:], oa[:])
        nc.vector.tensor_copy(od[:], ob[:])
        ot = otp.tile([P, QB], BF16, tag="ot", name="ot")
        ra = misc.tile([1, QB], F32, tag="ra", name="ra")
        rb = misc.tile([1, QB], F32, tag="rb", name="rb")
        bca = misc.tile([DK, QB], F32, tag="bca", name="bca")
        bcb = misc.tile([DK, QB], F32, tag="bcb", name="bcb")
        nc.vector.reciprocal(ra[:], oc[DK:VW, :])
        nc.gpsimd.partition_broadcast(bca[:], ra[:], channels=DK)
        nc.vector.tensor_mul(ot[0:DK, :], oc[0:DK, :], bca[:])
        nc.vector.reciprocal(rb[:], od[DK:VW, :])
        nc.gpsimd.partition_broadcast(bcb[:], rb[:], channels=DK)
        nc.vector.tensor_mul(ot[DK:P, :], od[0:DK, :], bcb[:])
        return ot

    with tc.tile_pool(name="wkv", bufs=1) as wkv, \
         tc.tile_pool(name="qtp", bufs=qtpb) as qtp, \
         tc.tile_pool(name="pexp", bufs=pexpb) as pexp, \
         tc.tile_pool(name="otp", bufs=NPAIR + 3) as otp, \
         tc.tile_pool(name="misc", bufs=2) as misc:

        # ---- DMA schedule (transfers serialize on the bus, so ORDER is
        # the lever): x window 0, then just the pair-0 slices of Wk/Wq so
        # K^T(0,0) and Q^T(0,0) unblock after ~5us, then Wv, biases, the
        # remaining Wk/Wq pairs, the other x windows, and Wo last.
        nc.sync.dma_start(xt3_w[0][:], xt_dram3[:, :, bass.ts(0, QB)])
        wk_p = [None] * NPAIR
        wq_p = [None] * NPAIR
        wk_p[0] = load_w_pair(wkv, "wk", dram["Wk"], 0)
        wq_p[0] = load_w_pair(wper, "wq", dram["Wq"], 0)
        wv_sb = load_w(wkv, "wv", dram["Wv"], HD)
        for pr in range(1, NPAIR):
            wk_p[pr] = load_w_pair(wkv, "wk", dram["Wk"], pr)
        for pr in range(1, NPAIR):
            wq_p[pr] = load_w_pair(wper, "wq", dram["Wq"], pr)
        for w in range(1, NQB):
            nc.sync.dma_start(xt3_w[w][:], xt_dram3[:, :, bass.ts(w, QB)])
        wo_sb = wper.tile([P, NPAIR * D], BF16, tag="wo", name="wo")
        nc.sync.dma_start(
            wo_sb[:].rearrange("p (r n) -> p r n", r=NPAIR),
            dram["Wo"].ap().rearrange("(r p) n -> p r n", p=P))

        def k_proj_items(w, pr):
            sl = bass.ts(w, QB)
            kps = ps_b.tile([P, QB], F32, tag="sp", name="kps")
            items = deque()
            for c in range(DC):
                def mm(c=c):
                    nc.tensor.matmul(kps[:],
                                     wk_p[pr][:, c * P:(c + 1) * P],
                                     xt3_w[w][:, c, :],
                                     start=(c == 0), stop=(c == DC - 1))
                    if c == DC - 1:
                        nc.vector.tensor_scalar_add(kt_tiles[pr][:, sl],
                                                    kps[:],
                                                    bk_sb[:, pr:pr + 1])
                items.append(mm)
            return items

        def v_proj_items(k):
            w, ki = divmod(k, QB // P)
            vps = ps_b.tile([P, HD], F32, tag="sp", name="vps")
            items = deque()
            for c in range(DC):
                def mm(c=c):
                    nc.tensor.matmul(vps[:],
                                     xt3_w[w][:, c, bass.ts(ki, P)],
                                     wslice(wv_sb, c, 0, HD),
                                     start=(c == 0), stop=(c == DC - 1))
                    if c == DC - 1:
                        v3 = v_tiles[k][:].rearrange("p (h v) -> p h v",
                                                     h=HLOC)
                        nc.vector.tensor_add(
                            v3[:, :, 0:DK],
                            vps[:].rearrange("p (h d) -> p h d", h=HLOC),
                            bv_sb[:].rearrange("p (h d) -> p h d", h=HLOC))
                        nc.vector.tensor_copy(v3[:, :, DK:VW],
                                              ones_sb[:].unsqueeze(2))
                items.append(mm)
            return items

        # ---- phase 1 merged with the (qb=0, pair=0) attention sweep:
        # per window, K^T(pair 0) is emitted up front (the sweep's scores
        # need it); V and K^T(pairs 1-3) ride the filler queue INSIDE the
        # sweep (filler_first) so the PE stays dense while the pair-0 exps
        # stream.  cap=14 drains exactly (32 V + 24 K) items per window.
        oa0 = ps_acc.tile([VW, QB], F32, tag="acc", name="oa")
        ob0 = ps_acc.tile([VW, QB], F32, tag="acc", name="ob")
        qt00 = None
        WPC = QB // P  # key chunks per window
        qt_next = qt_items = None
        for w in range(NQB):
            force(k_proj_items(w, 0))
            win_items = deque()
            for s4i in range(WPC):
                win_items.extend(v_proj_items(w * WPC + s4i))
            for pr in range(1, NPAIR):
                win_items.extend(k_proj_items(w, pr))
            queue_items(win_items, front=True)
            if w == 0:
                qt00, items00 = queue_qt_proj(0, 0)
                force(items00)
            la = ()
            if w == NQB - 1:
                # queue + force Q^T(0,1) now so the last window's sweep can
                # look ahead into pair 1's first score chunks.
                qt_next, qt_items = queue_qt_proj(0, 1)
                force(qt_items)
                la = (lambda: scores(kt_tiles[1], qt_next, 0),
                      lambda: scores(kt_tiles[1], qt_next, 1))
            sweep(kt_tiles[0], qt00, 0, oa0, ob0, w * WPC, (w + 1) * WPC,
                  lookahead=la, cap=14, filler_first=True)

        # ---- phase 2: remaining (qb, pair) sweeps, pipelined across
        # pair boundaries (scores lookahead + filler-spread projections) ----
        ot_tiles = [normalize(oa0, ob0)]
        seq = [(qb, pr) for qb in range(NQB) for pr in range(NPAIR)][1:]
        for idx, (qb, pr) in enumerate(seq):
            qt = qt_next
            force(qt_items)
            la = ()
            if idx + 1 < len(seq):
                nqb, npr = seq[idx + 1]
                qt_next, qt_items = queue_qt_proj(nqb, npr, front=True)
                nkt, nqt = kt_tiles[npr], qt_next
                la = (lambda nkt=nkt, nqt=nqt: scores(nkt, nqt, 0),
                      lambda nkt=nkt, nqt=nqt: scores(nkt, nqt, 1))
            oa = ps_acc.tile([VW, QB], F32, tag="acc", name="oa")
            ob = ps_acc.tile([VW, QB], F32, tag="acc", name="ob")
            sweep(kt_tiles[pr], qt, pr, oa, ob, 0, KC,
                  lookahead=la, prologue=False)
            ot_tiles.append(normalize(oa, ob))
            if pr == NPAIR - 1:
                queue_out_proj(qb, ot_tiles)
                ot_tiles = []
        drain_filler()


_CACHE = {}


def _get_nc():
    if "nc" not in _CACHE:
        _CACHE["nc"] = build_bass()
    return _CACHE["nc"]


def _bf16(a):
    import ml_dtypes
    return np.asarray(a, dtype=np.float32).astype(ml_dtypes.bfloat16)


def host_prep(x, Wq, bq, Wk, bk, Wv, bv, Wo, bo):
    """Build the 8 per-core input maps."""
    NPAIR = HL // 2
    NOC = D_MODEL // P
    in_maps = []
    for core in range(N_CORES):
        b, g = divmod(core, 2)
        lo, hi = g * HL * DK, (g + 1) * HL * DK
        in_maps.append({
            "xT": _bf16(np.ascontiguousarray(x[b].T)),
            "Wq": _bf16(Wq[:, lo:hi]),
            "Wk": _bf16(Wk[:, lo:hi]),
            "Wv": _bf16(Wv[:, lo:hi]),
            "Wo": _bf16(Wo[lo:hi, :]),
            "bq_t": np.ascontiguousarray(bq[lo:hi].reshape(NPAIR, P).T),
            "bk_t": np.ascontiguousarray(bk[lo:hi].reshape(NPAIR, P).T),
            "bv_bc": np.broadcast_to(bv[lo:hi], (P, HL * DK)).copy(),
            "bo_t": np.ascontiguousarray((bo * 0.5).reshape(NOC, P).T),
        })
    return in_maps


def host_gather(results):
    """Sum head-group partials and transpose back to [B, S, D]."""
    out = np.empty((BATCH, SEQ, D_MODEL), dtype=np.float32)
    for b in range(BATCH):
        yt = results[2 * b]["YT"] + results[2 * b + 1]["YT"]
        out[b] = yt.T
    return out


def kernel(x, Wq, bq, Wk, bk, Wv, bv, Wo, bo):
    nc = _get_nc()
    in_maps = host_prep(x, Wq, bq, Wk, bk, Wv, bv, Wo, bo)
    res = run_bass_kernel_spmd(nc, in_maps, core_ids=list(range(N_CORES)))
    return host_gather(res.results)
